# revision 3
# baseline (speedup 1.0000x reference)
"""Trainium2 Bass kernel (v11-series) for nn_IntraClassLoss (segment_reduce).

HW-measured 141.8us (prior best 165.8us; rel err 6.9e-05). Config:
 - targets host-cast to int16 (halves t DMA traffic; STT cost unchanged)
 - custom reciprocal_approx_fast writing bf16 directly (no cast op)
 - s1_c via STT accum_out (free), s2 classes 1,2 via ACT Square+accum,
   class 3 via plain ACT Square + PE ones-colsum into a PSUM
   accumulator over all chunks (default priority - do NOT deprioritize)
 - x DMAs issued before t DMA; exp split in two [128,2*size] halves
 - no gpsimd (its SBUF traffic halves concurrent DVE throughput)
 - deep buffers: io 5, e 4, den psum 3 (ramp chunk schedule tested
   and rejected: per-chunk overheads beat the fill/tail savings)

Engine steady-state per 1024-chunk: ACT ~8.0, DVE ~7.9, PE ~7.3,
DMA ~7.36.
"""

import os
import numpy as np
import ml_dtypes

import concourse.bass as bass
import concourse.bacc as bacc
import concourse.tile as tile
from concourse import mybir
from concourse.bass_utils import run_bass_kernel_spmd

F32 = mybir.dt.float32
BF16 = mybir.dt.bfloat16
I16 = mybir.dt.int16
AF = mybir.ActivationFunctionType
ALU = mybir.AluOpType

B, C, H, W = 16, 4, 1024, 1024
N_CORES = 8
B_LOC = B // N_CORES
P = 128
EPS = 1e-6
NCH = 1024  # tile capacity (max chunk size)


def _recip_fast(nc, out_ap, in_ap):
    from concourse.dve_ops import RECIP_APPROX_FAST_CONSTS, RECIPROCAL_APPROX_FAST
    c = RECIP_APPROX_FAST_CONSTS
    return nc.vector._custom_dve(
        RECIPROCAL_APPROX_FAST, out=out_ap, in0=in_ap,
        s0=c["s0"], s1=c["s1"], imm2=c["imm2"])


def _schedule(b_loc, free, ramp):
    """List of (b, off, size) chunks covering [0, free) per batch."""
    sched = []
    for b in range(b_loc):
        sizes = []
        if ramp and b == 0:
            sizes += [512, 512]
        rest = free - sum(sizes)
        tail = [512, 512] if (ramp and b == b_loc - 1) else []
        mid = rest - sum(tail)
        assert mid % NCH == 0
        sizes += [NCH] * (mid // NCH) + tail
        off = 0
        for s in sizes:
            sched.append((b, off, s))
            off += s
    return sched


def build_program(b_loc=B_LOC, h=H, w=W, io_bufs=5, e_bufs=4, den_bufs=3,
                  ramp=False):
    plane = h * w
    free = plane // P
    sched = _schedule(b_loc, free, ramp)
    n_sched = len(sched)

    nc = bacc.Bacc("TRN2", target_bir_lowering=False, debug=False)

    inputs_d = nc.dram_tensor("inputs", [b_loc, C, h, w], F32, kind="ExternalInput")
    targets_d = nc.dram_tensor("targets", [b_loc, h, w], I16, kind="ExternalInput")
    ident_d = nc.dram_tensor("ident", [P, P], BF16, kind="ExternalInput")
    ones_d = nc.dram_tensor("ones1", [P, 1], BF16, kind="ExternalInput")
    outs1_d = nc.dram_tensor("out_s1", [3, P, n_sched], F32, kind="ExternalOutput")
    outs2_d = nc.dram_tensor("out_s2", [2, P, n_sched], F32, kind="ExternalOutput")
    outs23_d = nc.dram_tensor("out_s23", [1, 512], F32, kind="ExternalOutput")

    with tile.TileContext(nc) as tc:
        with (
            tc.tile_pool(name="const", bufs=1) as constp,
            tc.tile_pool(name="io", bufs=io_bufs) as iop,
            tc.tile_pool(name="work", bufs=3) as workp,
            tc.tile_pool(name="stats", bufs=1) as statp,
            tc.tile_pool(name="psum", bufs=2, space="PSUM") as psump,
        ):
            ident = constp.tile([P, P], BF16)
            nc.sync.dma_start(ident[:], ident_d.ap())
            ones1 = constp.tile([P, 1], BF16)
            nc.sync.dma_start(ones1[:], ones_d.ap())

            s1_t = [
                statp.tile([P, n_sched], F32, tag=f"s1{ci}", name=f"s1{ci}")
                for ci in range(3)
            ]
            s2_t = [
                statp.tile([P, n_sched], F32, tag=f"s2{ci}", name=f"s2{ci}")
                for ci in range(2)
            ]
            s23p = psump.tile([1, 512], F32, tag="s23p", name="s23p", bufs=1)

            last_j = n_sched - 1
            for jj, (b, off, size) in enumerate(sched):
                sl = slice(off, off + size)
                nblk = size // 512

                xt = iop.tile([P, C * NCH], F32, tag="x", name="xt")
                for c in range(C):
                    x_src = inputs_d.ap()[b, c].rearrange(
                        "(p a) w -> p (a w)", p=P)
                    nc.sync.dma_start(
                        xt[:, c * size : (c + 1) * size], x_src[:, sl])

                t_tile = iop.tile([P, NCH], I16, tag="t", name="t_tile")
                tgt_ap = targets_d.ap()[b].rearrange("(p a) w -> p (a w)", p=P)
                nc.sync.dma_start(t_tile[:, :size], tgt_ap[:, sl])

                e = workp.tile([P, C * NCH], BF16, tag="e", name="e",
                               bufs=e_bufs)
                half = 2 * size
                nexp = int(os.environ.get("NEXP", "2"))
                step = C * size // nexp
                with tc.high_priority():
                    for q in range(nexp):
                        nc.scalar.activation(
                            e[:, q * step : (q + 1) * step],
                            xt[:, q * step : (q + 1) * step], AF.Exp)

                den = psump.tile([P, NCH], F32, tag="den", name="den",
                                 bufs=den_bufs)
                for c in range(C):
                    for i in range(nblk):
                        s2l = slice(i * 512, (i + 1) * 512)
                        nc.tensor.matmul(
                            den[:, s2l], ident[:],
                            e[:, c * size + i * 512 : c * size + (i + 1) * 512],
                            start=(c == 0), stop=(c == C - 1),
                        )

                rbf = workp.tile([P, NCH], BF16, tag="rbf", name="rbf")
                _recip_fast(nc, rbf[:, :size], den[:, :size])

                for ci, c in enumerate((1, 2, 3)):
                    ec = e[:, c * size : (c + 1) * size]
                    p_c = workp.tile([P, NCH], BF16, tag="p", name="p_c",
                                     bufs=6)
                    nc.vector.tensor_mul(p_c[:, :size], ec, rbf[:, :size])
                    if ci < 2:
                        g_c = workp.tile([P, NCH], BF16, tag="g", name="g_c",
                                         bufs=4)
                        nc.vector.scalar_tensor_tensor(
                            out=g_c[:, :size], in0=t_tile[:, :size], scalar=c,
                            in1=p_c[:, :size],
                            op0=ALU.is_equal, op1=ALU.mult,
                            accum_out=s1_t[ci][:, jj : jj + 1],
                        )
                        junk = workp.tile([P, NCH], BF16, tag="junk",
                                          name="junk", bufs=4)
                        with tc.high_priority(offset=-4000):
                            nc.scalar.activation(
                                junk[:, :size], g_c[:, :size], AF.Square,
                                accum_out=s2_t[ci][:, jj : jj + 1],
                            )
                    else:
                        g_c = workp.tile([P, NCH], BF16, tag="g3", name="g3",
                                         bufs=3)
                        nc.vector.scalar_tensor_tensor(
                            out=g_c[:, :size], in0=t_tile[:, :size], scalar=c,
                            in1=p_c[:, :size],
                            op0=ALU.is_equal, op1=ALU.mult,
                            accum_out=s1_t[ci][:, jj : jj + 1],
                        )
                        junk3 = workp.tile([P, NCH], BF16, tag="junk3",
                                           name="junk3", bufs=3)
                        with tc.high_priority(offset=-4000):
                            nc.scalar.activation(
                                junk3[:, :size], g_c[:, :size], AF.Square)
                        for blk in range(nblk):
                            nc.tensor.matmul(
                                s23p[:, :], ones1[:],
                                junk3[:, blk * 512 : (blk + 1) * 512],
                                start=(jj == 0 and blk == 0),
                                stop=(jj == last_j and blk == nblk - 1),
                                skip_group_check=True,
                            )

            for ci in range(3):
                nc.sync.dma_start(outs1_d.ap()[ci], s1_t[ci][:])
            for ci in range(2):
                nc.sync.dma_start(outs2_d.ap()[ci], s2_t[ci][:])
            s23s = statp.tile([1, 512], F32, tag="s23s", name="s23s")
            nc.vector.tensor_copy(s23s[:], s23p[:])
            nc.sync.dma_start(outs23_d.ap(), s23s[:])

    nc.compile()
    return nc, n_sched


_CACHED = {}


def _get_program():
    if "nc" not in _CACHED:
        kw = {}
        if os.environ.get("RAMP"):
            kw["ramp"] = True
        _CACHED["nc"] = build_program(**kw)[0]
    return _CACHED["nc"], None


def make_in_maps(inputs, targets):
    ident = np.eye(P, dtype=ml_dtypes.bfloat16)
    ones1 = np.ones((P, 1), dtype=ml_dtypes.bfloat16)
    t16 = targets.astype(np.int16)
    return [
        {
            "inputs": np.ascontiguousarray(inputs[i * B_LOC : (i + 1) * B_LOC]),
            "targets": np.ascontiguousarray(t16[i * B_LOC : (i + 1) * B_LOC]),
            "ident": ident,
            "ones1": ones1,
        }
        for i in range(N_CORES)
    ]


def finish_host(results, cnt):
    s1 = np.zeros(3, dtype=np.float64)
    s2 = np.zeros(3, dtype=np.float64)
    for r in results:
        s1 += r["out_s1"].astype(np.float64).sum(axis=(1, 2))
        s2[:2] += r["out_s2"].astype(np.float64).sum(axis=(1, 2))
        s2[2] += r["out_s23"].astype(np.float64).sum()
    mean = s1 / (cnt + EPS)
    var = (s2 - 2.0 * mean * s1 + cnt * mean * mean) / (cnt + EPS)
    intra = np.where(cnt > 0, var, 0.0).sum()
    return np.float32(intra / (C - 1))


def kernel(inputs: np.ndarray, targets: np.ndarray) -> np.ndarray:
    nc, _ = _get_program()
    in_maps = make_in_maps(inputs, targets)
    res = run_bass_kernel_spmd(nc, in_maps, list(range(N_CORES)))
    cnt = np.bincount(targets.ravel(), minlength=C)[1:C].astype(np.float64)
    return finish_host([res.results[i] for i in range(N_CORES)], cnt)
